# revision 72
# baseline (speedup 1.0000x reference)
"""BackflowNet GNN message-passing kernel for 8x Trainium2 NeuronCores.

Data-parallel over the walker axis B=128 -> 16 walkers per core, processed as
8 "pairs" (2 walkers block-diag-packed into the 128-partition dim).

Math restructuring (exact, host-side weight folding only):
  he0 = gelu(ein @ ew1 + eb1) @ ew2 + eb2           -> keep g_e = gelu(...)
  layer l: z = hv[:,i]@Wtop + he@Wbot + b1          (he = prev g @ w2 + b2 folded)
           g = gelu(z); he' = g @ w2 + b2
           m = gelu(he' @ e2v_w1 + e2v_b1)          (per-edge)
           hv += (sum_{i!=j} m_i) @ e2v_w2/(N-1) + e2v_b2   (sum moved before w2)
  head: dx = tanh(hv@hw1+hb1) @ (hw2*sp) + hb2*sp;  out = dx - mean_j dx
  hu1 shortcut: hu1 = wt1.T@hv1 = wt1.T@hv0 (early) + (wg0@wt1).T@accd0, with
  hv1's bias folded into the layer-1 gelu bias - stage3 starts straight off
  the aggregation instead of the serial hv1 -> hu1 chain.
Per-edge tensors are feature-major [feat(part), e] with e = j*64 + i
(i = source = innermost so the aggregation is an innermost DVE reduce).

Everything the PE touches is bf16 (weights, activation streams): 1 col/cycle
like f32r but with FWL fast weight loads, half the SBUF, and standalone
Ldweights+Matmult lowering whose redundant weight reloads _dedup_ldw elides
at the BIR level (walrus --enable-ldw-opt rejects standalone InstLdweights).
PSUM accumulates fp32; biases/x-packs stay fp32. The i<-j "hu" broadcast-add
is folded into the PE as an identity-broadcast accumulation (eye_bc) rather
than a DVE add into PSUM (which runs 1x and sat on the critical path).

Scheduling: 5 pipeline stages (DMA prefetch / ein+wc / agg0 / wb1 / agg1+head)
emitted as block-granular thunks, round-robined RRN=2 at a time across the
4-5 pairs in flight so each engine's queue interleaves independent work; the
last pair's stage3/4 are fused block-wise to shorten the drain tail.
"""

import math
import os

import numpy as np

import concourse.bass as bass
import concourse.mybir as mybir
import concourse.tile as tile
import concourse.bass_utils as _bu
from concourse.bass_utils import run_bass_kernel_spmd

if os.environ.get("BACKFLOW_LDW_OPT", "0") == "1" and not getattr(_bu, "_ldw_patched", False):
    _bu._ldw_patched = True
    _orig_run = _bu.run_command

    def _run(cmd, cwd=None):
        if cmd and "walrus_driver" in cmd[0]:
            cmd = [c if c != "--enable-ldw-opt=false" else "--enable-ldw-opt=true"
                   for c in cmd]
        return _orig_run(cmd, cwd=cwd)

    _bu.run_command = _run

NCORES = 8
B, N, D = 128, 64, 2
H = 64
M = 64
BC = B // NCORES          # walkers per core
PAIRS = BC // 2           # walker pairs per core
E = N * N                 # edges (incl. diagonal) per walker
SUB = int(os.environ.get("BACKFLOW_SUB", "512"))  # matmul moving free dim
                          # (bf16 moving operands allow up to 1024)
# gelu/psum blocks: uneven (3 ACT ops per pass instead of 4 cuts the
# per-op (N+352)/1.2 ns ACT overhead)
_B = int(os.environ.get("BACKFLOW_BLK", "1024"))
BLOCKS = ([(0, 1536), (1536, 1536), (3072, 1024)] if _B == 1536
          else [(0, 1024), (1024, 1024), (2048, 1024), (3072, 1024)])
PBLK = _B                 # psum tile width
# emission groups: pairs of blocks share one weight-load train per src
# (measured worse: holding two PSUM tiles per group starves the pipeline)
BPAIR = os.environ.get("BACKFLOW_BPAIR", "0") == "1"
BGROUPS = ([(0, 1), (2, 3)] if (len(BLOCKS) == 4 and BPAIR)
           else [(b,) for b in range(len(BLOCKS))])
F32 = mybir.dt.float32
F32R = mybir.dt.float32r
BF16 = mybir.dt.bfloat16
AF = mybir.ActivationFunctionType
AX = mybir.AxisListType

HU_DVE = os.environ.get("BACKFLOW_HU_DVE", "0") == "1"
# agg-pass gelu outputs + row-sum accumulators in bf16: DVE tensor_reduce
# runs in 2x mode (16-bit, step 1) halving the aggregation reduce cost
BF16_AGG = os.environ.get("BACKFLOW_BF16_AGG", "1") == "1"
GB_DT = BF16 if BF16_AGG else F32R

_BUILT = {}


def _legalize_sync(bir_bytes):
    """Walrus on this toolchain encodes at most one semaphore wait per
    engine instruction (none on DMA queue entries). Tile attaches as many
    waits as deps require, so spill the surplus into standalone
    EventSemaphore instructions on the same engine, placed just before."""
    import json as _json

    d = _json.loads(bir_bytes)
    n = [0]
    # salt the generated names so compile-flag-only changes (which do not
    # otherwise alter the BIR/HLO bytes) miss the remote executable cache
    salt = "L" + os.environ.get("BACKFLOW_LDW_OPT", "0") + os.environ.get(
        "BACKFLOW_SALT", ""
    )

    def fix_block(bb):
        insts = bb.get("instructions")
        if not insts:
            return
        out = []
        for ins in insts:
            si = ins.get("sync_info")
            waits = (si or {}).get("on_wait") or []
            opc = ins.get("opcode", "")
            if opc == "EventSemaphore":
                allowed = 1
            elif opc.startswith("DMA") or ins.get("queue"):
                allowed = 0
            else:
                allowed = 1
            if len(waits) > allowed:
                keep, spill = waits[:allowed], waits[allowed:]
                for w in spill:
                    n[0] += 1
                    out.append({
                        "debug": ins.get("debug", 0),
                        "engine": ins["engine"],
                        "ins": [],
                        "outs": [],
                        "name": f"evw{salt}-{n[0]}",
                        "opcode": "EventSemaphore",
                        "sync_info": {"on_update": [], "on_wait": [w]},
                    })
                si["on_wait"] = keep
            out.append(ins)
        bb["instructions"] = out

    def walk(obj):
        if isinstance(obj, dict):
            if "instructions" in obj:
                fix_block(obj)
            else:
                for v in obj.values():
                    walk(v)
        elif isinstance(obj, list):
            for v in obj:
                walk(v)

    walk(d)
    return _json.dumps(d).encode()


def _dedup_ldw(bir_bytes):
    """bf16 matmuls lower to standalone Ldweights+Matmult pairs with one
    weight load per matmul. The PE array retains stationary weights across
    matmuls, so consecutive PE loads of the identical weight AP are
    redundant — drop them (walrus's --enable-ldw-opt rejects standalone
    InstLdweights, so do it here). Dropped loads' semaphore waits are
    spilled as standalone PE EventSemaphores to preserve ordering."""
    import json as _json

    d = _json.loads(bir_bytes)
    n = [0]

    def fix_block(bb):
        insts = bb.get("instructions")
        if not insts:
            return
        out = []
        last_sig = None
        pending = []
        for ins in insts:
            if ins.get("engine") == "PE":
                opc = ins.get("opcode", "")
                if opc == "Ldweights":
                    sig = _json.dumps(
                        [
                            ins.get("ins"),
                            ins.get("tile_position"),
                            ins.get("tile_size"),
                            ins.get("perf_mode"),
                            ins.get("is_transpose"),
                        ],
                        sort_keys=True,
                    )
                    si = ins.get("sync_info") or {}
                    if sig == last_sig and not si.get("on_update"):
                        pending.extend(si.get("on_wait") or [])
                        continue
                    last_sig = sig
                elif opc not in ("Matmult", "EventSemaphore"):
                    last_sig = None
                if pending:
                    for w in pending:
                        n[0] += 1
                        out.append({
                            "debug": ins.get("debug", 0),
                            "engine": "PE",
                            "ins": [],
                            "outs": [],
                            "name": f"ldww-{n[0]}",
                            "opcode": "EventSemaphore",
                            "sync_info": {"on_update": [], "on_wait": [w]},
                        })
                    pending = []
            out.append(ins)
        bb["instructions"] = out

    def walk(obj):
        if isinstance(obj, dict):
            if "instructions" in obj:
                fix_block(obj)
            else:
                for v in obj.values():
                    walk(v)
        elif isinstance(obj, list):
            for v in obj:
                walk(v)

    walk(d)
    return _json.dumps(d).encode()


def _build():
    nc = bass.Bass(
        "TRN2", target_bir_lowering=False, debug=False, enable_asserts=False
    )

    # Constants in two packed dram tensors: bf16 weights/streams (FWL-fast
    # weight loads, half the SBUF/DMA), fp32 biases + x packs (ACT bias APs
    # and fp32 phase-0 DVE math).
    # wpb (bf16) column map:
    #  0:128    wn1 [6,128]       128:256  wn2        256:384  wein [8,128]
    #  384:512  wc                512:640  wa0        640:768  wb1
    #  768:896  wa1               896:1024 wg0        1024:1152 wg1
    #  1152:1280 wt0              1280:1408 wt1       1408:1536 wh1
    #  1536:1540 wh2 [128,4]      1540:1604 eye [64,64]
    #  1604:2116 nin [6,512]      2116:2244 wtg
    # wpf (fp32) column map:
    #  0:12 biases  12:28 xp_lhs  28:156 xp_rhs  156:172 xq_lhs  172:236 xq_rhs
    CB, CF = 2244, 236
    wpack_bf = nc.dram_tensor("wpb", [128, CB], BF16, kind="ExternalInput").ap()
    wpack_f32 = nc.dram_tensor("wpf", [128, CF], F32, kind="ExternalInput").ap()
    out_dx = nc.dram_tensor("out_dx", [BC, N, D], F32, kind="ExternalOutput").ap()

    with tile.TileContext(nc) as tc:
        with (
            tc.tile_pool(name="consts", bufs=1) as consts,
            tc.tile_pool(name="feat", bufs=1) as feat,
            tc.tile_pool(name="big", bufs=12) as big,
            tc.tile_pool(name="gblk", bufs=10) as gblk,
            tc.tile_pool(name="small", bufs=24) as small,
            tc.tile_pool(name="swide", bufs=4) as swide,
            tc.tile_pool(name="pbig", bufs=(2 if _B == 1536 else 3),
                         space="PSUM") as pbig,
            tc.tile_pool(name="psmall", bufs=2, space="PSUM") as psmall,
        ):
            wpb = consts.tile([128, CB], BF16, name="wpb", tag="wpb")
            wpf = consts.tile([128, CF], F32, name="wpf", tag="wpf")
            # phase-0 inputs + node-MLP columns land first on their own
            # queue so compute starts before the big weight block finishes
            nc.gpsimd.dma_start(out=wpf, in_=wpack_f32)
            nc.gpsimd.dma_start(out=wpb[:, 1604:CB], in_=wpack_bf[:, 1604:CB])
            nc.sync.dma_start(out=wpb[:, 0:1604], in_=wpack_bf[:, 0:1604])
            wn1 = wpb[0:6, 0:128]
            wn2 = wpb[:, 128:256]
            wein = wpb[0:8, 256:384]
            wc = wpb[:, 384:512]
            wa0 = wpb[:, 512:640]
            wb1 = wpb[:, 640:768]
            wa1 = wpb[:, 768:896]
            wg0 = wpb[:, 896:1024]
            wg1 = wpb[:, 1024:1152]
            wt0 = wpb[:, 1152:1280]
            wt1 = wpb[:, 1280:1408]
            wh1 = wpb[:, 1408:1536]
            wh2 = wpb[:, 1536:1540]
            eye_t = wpb[0:64, 1540:1604]
            nin = wpb[0:6, 1604:2116]
            wtg = wpb[:, 2116:2244]
            bia = wpf[:, 0:12]
            xpl = wpf[:, 12:28]
            xpr = wpf[:, 28:156]
            xql = wpf[:, 156:172]
            xqr = wpf[:, 172:236]
            eps_t = consts.tile([128, 1], F32, name="eps_t", tag="eps_t")
            nc.vector.memset(eps_t, 1e-12)

            # identity-broadcast rhs: rhs[k,(j,i)] = eye[k,i], j in 8-block
            eye_bc = eye_t.unsqueeze(1).broadcast_to([64, SUB // N, 64])


            import contextlib
            rep = int(os.environ.get("BACKFLOW_REPEAT", "1"))
            loop_cm = tc.For_i(0, rep, 1) if rep > 1 else contextlib.nullcontext()
            with loop_cm:
                # -------- phase 0: dr / r2 / rr in packed layouts ---------------
                # (w,d,jq2)-packed dr for the ein rows (contiguous per pair)
                dre_pk = feat.tile([128, 1024], BF16, name="dre_pk", tag="drepk")
                nc.vector.tensor_sub(
                    out=dre_pk.rearrange("p (j i) -> p j i", i=N),
                    in0=xql.unsqueeze(2).broadcast_to([128, 16, N]),
                    in1=xqr.unsqueeze(1).broadcast_to([128, 16, N]),
                )
                # (w,jq)-packed dr for r2/rr
                dr_pk = feat.tile([128, 1024], F32, name="dr_pk", tag="drpk")
                nc.vector.tensor_sub(
                    out=dr_pk.rearrange("p (d j i) -> p d j i", d=2, i=N),
                    in0=xpl.rearrange("p (d j) -> p d j", d=2)
                        .unsqueeze(3).broadcast_to([128, 2, 8, N]),
                    in1=xpr.rearrange("p (d i) -> p d i", d=2)
                        .unsqueeze(2).broadcast_to([128, 2, 8, N]),
                )
                sq_pk = feat.tile([128, 1024], F32, name="sq_pk", tag="sqpk")
                nc.vector.tensor_mul(out=sq_pk, in0=dr_pk, in1=dr_pk)
                sqv = sq_pk.rearrange("p (d f) -> p d f", d=2)
                r2_pk = feat.tile([128, 512], BF16, name="r2_pk", tag="r2pk")
                nc.vector.tensor_add(out=r2_pk, in0=sqv[:, 0, :], in1=sqv[:, 1, :])
                rr_pk = feat.tile([128, 512], BF16, name="rr_pk", tag="rrpk")
                sqrt_inst = nc.scalar.activation(
                    out=rr_pk, in_=r2_pk, func=AF.Sqrt, bias=eps_t, scale=1.0
                )

                # -------- batched node MLP (all 8 pairs) ------------------------
                zn = psmall.tile([128, 8 * N], F32, name="zn", tag="ps_s")
                zn_inst = nc.tensor.matmul(zn, wn1, nin)
                from concourse.tile_rust import add_dep_helper
                add_dep_helper(zn_inst.ins, sqrt_inst.ins,
                               reason="keep sqrt first in ACT stream (table set)")
                gn = swide.tile([128, 8 * N], BF16, name="gn", tag="sw")
                nc.scalar.activation(
                    out=gn, in_=zn, func=AF.Gelu, bias=bia[:, 0:1], scale=1.0
                )
                hv0p = psmall.tile([128, 8 * N], F32, name="hv0p", tag="ps_s")
                nc.tensor.matmul(hv0p, wn2, gn)
                hv0a = swide.tile([128, 8 * N], BF16, name="hv0a", tag="sw")
                nc.vector.tensor_scalar_add(out=hv0a, in0=hv0p, scalar1=bia[:, 1:2])

                # -------- software-pipelined per-pair stages --------------------
                st = [dict() for _ in range(PAIRS)]

                def edge_pass_group(bls, dsts, srcs, gelu_bias, hu_m=None,
                                    post=None):
                    """Emit a group of blocks sharing one weight-load train
                    per src (srcs outer, blocks+chunks inner: consecutive
                    same-weight matmuls keep the PE array's stationary
                    weights, and the bf16 standalone-LDW dedup drops the
                    reloads). `post(bi, bl)` runs per block after its gelu
                    (reduce/sub for agg passes)."""
                    pss = []
                    for _ in bls:
                        pss.append(
                            pbig.tile([128, PBLK], F32, name="ps", tag="ps_b")
                        )
                    for k, (lhsT, rhs_fn) in enumerate(srcs):
                        for bi, bl in enumerate(bls):
                            lo0, blen = BLOCKS[bl]
                            for s in range(blen // SUB):
                                lo = lo0 + s * SUB
                                out_sl = pss[bi][:, s * SUB : (s + 1) * SUB]
                                nc.tensor.matmul(
                                    out_sl,
                                    lhsT,
                                    rhs_fn(lo),
                                    start=(k == 0),
                                    stop=(k == len(srcs) - 1),
                                )
                    for bi, bl in enumerate(bls):
                        lo0, blen = BLOCKS[bl]
                        ps = pss[bi]
                        if hu_m is not None:
                            psv = ps[:, 0:blen].rearrange("p (j i) -> p j i", i=N)
                            nc.vector.tensor_add(
                                out=psv, in0=psv,
                                in1=hu_m.unsqueeze(1).broadcast_to(
                                    [128, blen // N, N]
                                ),
                            )
                        nc.scalar.activation(
                            out=dsts[bi][:, 0:blen], in_=ps[:, 0:blen],
                            func=AF.Gelu, bias=gelu_bias, scale=1.0,
                        )
                        if post is not None:
                            post(bi, bl)

                def edge_pass_blk(bl, dst, srcs, gelu_bias, hu_m=None):
                    edge_pass_group([bl], [dst], srcs, gelu_bias, hu_m=hu_m)

                def agg_group(bls, src_g, wagg, bias_col, acc, accd):
                    gbs = [
                        gblk.tile([128, PBLK], GB_DT, name="gb", tag="gb")
                        for _ in bls
                    ]

                    def post(bi, bl):
                        lo0, blen = BLOCKS[bl]
                        gb = gbs[bi]
                        jlo = lo0 // N
                        jb = blen // N
                        with nc.allow_low_precision(
                            reason="bf16 row-sum: 2e-2 rel-err budget"
                        ):
                            nc.vector.reduce_sum(
                                out=acc[:, jlo : jlo + jb],
                                in_=gb[:, 0:blen].rearrange(
                                    "p (j i) -> p j i", i=N
                                ),
                                axis=AX.X,
                            )
                        diag = gb[:, jlo : jlo + (jb - 1) * (N + 1) + 1 : N + 1]
                        nc.vector.tensor_sub(
                            out=accd[:, jlo : jlo + jb],
                            in0=acc[:, jlo : jlo + jb],
                            in1=diag,
                        )

                    edge_pass_group(
                        bls, gbs,
                        [(wagg, lambda lo: src_g[:, lo : lo + SUB])],
                        bias_col, post=post,
                    )

                # Each stage returns a list of block-granular thunks; the
                # scheduler round-robins the lists so every engine's queue
                # interleaves independent work from 4-5 pairs at block
                # granularity (one stage's stall no longer head-of-line
                # blocks the whole step).
                def stage0(p):
                    # prefetch: per-pair edge-input DMAs one pipeline step
                    # early so stage1's matmuls never head-of-line block PE
                    def t_dma():
                        s = st[p]
                        ein = big.tile([8, E], BF16, name="ein", tag="stream")
                        s["ein"] = ein
                        nc.gpsimd.dma_start(
                            out=ein[0:4, :], in_=dre_pk[16 * p : 16 * p + 16, :]
                        )
                        nc.gpsimd.dma_start(
                            out=ein[4:6, :], in_=r2_pk[16 * p : 16 * p + 16, :]
                        )
                        nc.gpsimd.dma_start(
                            out=ein[6:8, :], in_=rr_pk[16 * p : 16 * p + 16, :]
                        )
                    return [t_dma]

                def stage1(p):
                    s = st[p]

                    def t_setup():
                        hv0 = hv0a[:, p * N : (p + 1) * N]
                        s["hv0"] = hv0
                        if HU_DVE:
                            hu0p = psmall.tile([128, 64], F32, name="hu0p", tag="ps_s")
                            nc.tensor.matmul(hu0p, wt0, hv0)
                            hu0 = small.tile([128, 64], F32, name="hu0", tag="sm")
                            nc.vector.tensor_copy(out=hu0, in_=hu0p)
                            # early half of hu1: wt1.T @ hv0 (the accd0-
                            # dependent half lands in stage2 via wtg)
                            hu1ap = psmall.tile([128, 64], F32, name="hu1ap", tag="ps_s")
                            nc.tensor.matmul(hu1ap, wt1, hv0)
                            hu1a = small.tile([128, 64], F32, name="hu1a", tag="sm")
                            nc.vector.tensor_copy(out=hu1a, in_=hu1ap)
                        else:
                            hu0p = psmall.tile([64, 128], F32, name="hu0p", tag="ps_s")
                            nc.tensor.matmul(hu0p, hv0, wt0)
                            hu0 = small.tile([64, 128], BF16, name="hu0", tag="smh")
                            nc.vector.tensor_copy(out=hu0, in_=hu0p)
                            hu1ap = psmall.tile([64, 128], F32, name="hu1ap", tag="ps_s")
                            nc.tensor.matmul(hu1ap, hv0, wt1)
                            hu1a = small.tile([64, 128], F32, name="hu1a", tag="sm")
                            nc.vector.tensor_copy(out=hu1a, in_=hu1ap)
                        s["hu0"] = hu0
                        s["hu1a"] = hu1a
                        s["g1"] = big.tile([128, E], BF16, name="g1", tag="stream")
                        s["gebs"] = {}

                    def t_ein(gi):
                        bls = BGROUPS[gi]
                        gebs = []
                        for bl in bls:
                            # own tag: under RR interleave, 'gb'-tag tiles
                            # from other stages would recycle geb's buffer
                            # before the wc pass (a later PE instruction)
                            # reads it — same-queue wait inversion = deadlock
                            geb = gblk.tile([128, PBLK], BF16, name="geb",
                                            tag="geb", bufs=5)
                            s["gebs"][bl] = geb
                            gebs.append(geb)
                        ein = s["ein"]
                        edge_pass_group(
                            list(bls), gebs,
                            [(wein, lambda lo: ein[:, lo : lo + SUB])],
                            bia[:, 2:3],
                        )

                    def t_wc(gi):
                        bls = BGROUPS[gi]
                        g1 = s["g1"]
                        dsts = []
                        for bl in bls:
                            lo0, blen = BLOCKS[bl]
                            dsts.append(g1[:, lo0 : lo0 + blen])

                        def geb_rhs(lo):
                            bl = next(
                                b for b in bls
                                if BLOCKS[b][0] <= lo < BLOCKS[b][0] + BLOCKS[b][1]
                            )
                            lo0 = BLOCKS[bl][0]
                            return s["gebs"][bl][:, lo - lo0 : lo - lo0 + SUB]

                        if HU_DVE:
                            edge_pass_group(
                                list(bls), dsts,
                                [(wc, geb_rhs)],
                                bia[:, 3:4], hu_m=s["hu0"],
                            )
                        else:
                            edge_pass_group(
                                list(bls), dsts,
                                [
                                    (wc, geb_rhs),
                                    (s["hu0"], lambda lo: eye_bc),
                                ],
                                bia[:, 3:4],
                            )

                    import functools
                    return (
                        [t_setup]
                        + [functools.partial(t_ein, g) for g in range(len(BGROUPS))]
                        + [functools.partial(t_wc, g) for g in range(len(BGROUPS))]
                    )

                def stage2(p):
                    s = st[p]

                    def t_setup():
                        s["acc0"] = small.tile([128, N], GB_DT, name="acc0", tag="smh")
                        s["accd0"] = small.tile([128, N], BF16, name="accd0", tag="smh")

                    def t_blk(gi):
                        agg_group(list(BGROUPS[gi]), s["g1"], wa0, bia[:, 4:5],
                                  s["acc0"], s["accd0"])

                    def t_tail():
                        accd0 = s["accd0"]
                        # critical: hu1 = hu1a + wtg.T@accd0 feeds stage3
                        if HU_DVE:
                            hu1cp = psmall.tile([128, 64], F32, name="hu1cp", tag="ps_s")
                            nc.tensor.matmul(hu1cp, wtg, accd0)
                            hu1 = small.tile([128, 64], F32, name="hu1", tag="sm")
                        else:
                            hu1cp = psmall.tile([64, 128], F32, name="hu1cp", tag="ps_s")
                            nc.tensor.matmul(hu1cp, accd0, wtg)
                            hu1 = small.tile([64, 128], BF16, name="hu1", tag="smh")
                        nc.vector.tensor_add(out=hu1, in0=hu1cp, in1=s["hu1a"])
                        s["hu1"] = hu1
                        # off the stage3 critical path: hv1 (used by stage4)
                        u0p = psmall.tile([128, N], F32, name="u0p", tag="ps_s")
                        nc.tensor.matmul(u0p, wg0, accd0)
                        hv1 = small.tile([128, N], F32R, name="hv1", tag="sm")
                        nc.vector.scalar_tensor_tensor(
                            out=hv1, in0=u0p, scalar=bia[:, 7:8], in1=s["hv0"],
                            op0=mybir.AluOpType.add, op1=mybir.AluOpType.add,
                        )
                        s["hv1"] = hv1

                    import functools
                    return (
                        [t_setup]
                        + [functools.partial(t_blk, g) for g in range(len(BGROUPS))]
                        + [t_tail]
                    )

                def stage3(p):
                    s = st[p]

                    def t_setup():
                        s["g3"] = big.tile([128, E], BF16, name="g3", tag="stream")

                    def t_blk(gi):
                        bls = BGROUPS[gi]
                        g1, g3 = s["g1"], s["g3"]
                        dsts = []
                        for bl in bls:
                            lo0, blen = BLOCKS[bl]
                            dsts.append(g3[:, lo0 : lo0 + blen])
                        if HU_DVE:
                            edge_pass_group(
                                list(bls), dsts,
                                [(wb1, lambda lo: g1[:, lo : lo + SUB])],
                                bia[:, 5:6], hu_m=s["hu1"],
                            )
                        else:
                            edge_pass_group(
                                list(bls), dsts,
                                [
                                    (wb1, lambda lo: g1[:, lo : lo + SUB]),
                                    (s["hu1"], lambda lo: eye_bc),
                                ],
                                bia[:, 5:6],
                            )

                    import functools
                    return [t_setup] + [
                        functools.partial(t_blk, g) for g in range(len(BGROUPS))
                    ]

                def stage4(p):
                    s = st[p]

                    def t_setup():
                        s["acc1"] = small.tile([128, N], GB_DT, name="acc1", tag="smh")
                        s["accd1"] = small.tile([128, N], BF16, name="accd1", tag="smh")

                    def t_blk(gi):
                        agg_group(list(BGROUPS[gi]), s["g3"], wa1, bia[:, 6:7],
                                  s["acc1"], s["accd1"])

                    def t_tail():
                        accd1 = s["accd1"]
                        u1p = psmall.tile([128, N], F32, name="u1p", tag="ps_s")
                        nc.tensor.matmul(u1p, wg1, accd1)
                        hv2 = small.tile([128, N], BF16, name="hv2", tag="smh")
                        nc.vector.scalar_tensor_tensor(
                            out=hv2, in0=u1p, scalar=bia[:, 8:9], in1=s["hv1"],
                            op0=mybir.AluOpType.add, op1=mybir.AluOpType.add,
                        )
                        thp = psmall.tile([128, N], F32, name="thp", tag="ps_s")
                        nc.tensor.matmul(thp, wh1, hv2)
                        th = small.tile([128, N], BF16, name="th", tag="smh")
                        nc.scalar.activation(
                            out=th, in_=thp, func=AF.Tanh, bias=bia[:, 9:10], scale=1.0
                        )
                        dxp = psmall.tile([4, N], F32, name="dxp", tag="ps_s")
                        nc.tensor.matmul(dxp, wh2, th)
                        dx = small.tile([4, N], F32, name="dx", tag="sm")
                        nc.vector.tensor_scalar_add(
                            out=dx, in0=dxp, scalar1=bia[0:4, 10:11]
                        )
                        msum = small.tile([4, 1], F32, name="msum", tag="sm1")
                        nc.vector.reduce_sum(out=msum, in_=dx, axis=AX.X)
                        negm = small.tile([4, 1], F32, name="negm", tag="sm1")
                        nc.vector.tensor_scalar_mul(
                            out=negm, in0=msum, scalar1=-1.0 / N
                        )
                        dxf = small.tile([4, N], F32, name="dxf", tag="sm")
                        nc.vector.tensor_scalar_add(out=dxf, in0=dx, scalar1=negm)
                        nc.sync.dma_start(
                            out=out_dx[2 * p].transpose([1, 0]), in_=dxf[0:2, :]
                        )
                        nc.sync.dma_start(
                            out=out_dx[2 * p + 1].transpose([1, 0]), in_=dxf[2:4, :]
                        )

                    import functools
                    return (
                        [t_setup]
                        + [functools.partial(t_blk, g) for g in range(len(BGROUPS))]
                        + [t_tail]
                    )

                stages = [stage0, stage1, stage2, stage3, stage4]
                order = [int(c) for c in os.environ.get("BACKFLOW_ORDER", "01243")]
                # round-robin grain: thunks popped per stage per cycle.
                # 0 = no interleave (whole stage at once)
                RRN = int(os.environ.get("BACKFLOW_RRN", "2"))
                for t in range(PAIRS + len(stages) - 1):
                    lists = []
                    for si in order:
                        p = t - si
                        if 0 <= p < PAIRS:
                            if si == 3 and p == PAIRS - 1:
                                # drain tail: fuse the last pair's stage3/4
                                # block-wise so they pipeline instead of
                                # running back-to-back alone
                                l3, l4 = stage3(p), stage4(p)
                                fused = [l3.pop(0), l3.pop(0), l4.pop(0)]
                                while l3 or l4:
                                    if l3:
                                        fused.append(l3.pop(0))
                                    if l4:
                                        fused.append(l4.pop(0))
                                lists.append(fused)
                            elif si == 4 and p == PAIRS - 1:
                                pass
                            else:
                                lists.append(stages[si](p))
                    if RRN > 0:
                        while any(lists):
                            for lst in lists:
                                for _ in range(min(RRN, len(lst))):
                                    lst.pop(0)()
                    else:
                        for lst in lists:
                            for th in lst:
                                th()

    patched = _legalize_sync(_dedup_ldw(nc.to_json_bytes()))
    nc.to_json_bytes = lambda: patched
    return nc


def _prep_weights(inputs):
    f8 = np.float64
    g = {k: np.asarray(v, dtype=f8) for k, v in inputs.items()}
    inv = 1.0 / (N - 1)

    wtop0 = g["v2e_w1"][0][:H]
    wbot0 = g["v2e_w1"][0][H:]
    wtop1 = g["v2e_w1"][1][:H]
    wbot1 = g["v2e_w1"][1][H:]

    w_c = g["edge_w2"] @ wbot0
    b_p2 = g["edge_b2"] @ wbot0 + g["v2e_b1"][0]
    w_a0 = g["v2e_w2"][0] @ g["e2v_w1"][0]
    b_p3 = g["v2e_b2"][0] @ g["e2v_w1"][0] + g["e2v_b1"][0]
    w_b1 = g["v2e_w2"][0] @ wbot1
    # hv1's bias (e2v_b2[0]) contribution to hu1 = hv1.T @ wtop1 is constant
    # over i, so it folds into the layer-1 gelu bias column
    b_p4 = g["v2e_b2"][0] @ wbot1 + g["v2e_b1"][1] + g["e2v_b2"][0] @ wtop1
    w_a1 = g["v2e_w2"][1] @ g["e2v_w1"][1]
    b_p5 = g["v2e_b2"][1] @ g["e2v_w1"][1] + g["e2v_b1"][1]
    w_g0 = g["e2v_w2"][0] * inv
    w_g1 = g["e2v_w2"][1] * inv
    sp = float(np.log1p(np.exp(g["scale"][0])))
    w_h2 = g["head_w2"] * sp
    b_h2 = g["head_b2"] * sp

    def bd(w):  # [64,64] -> [128,128] block-diag
        o = np.zeros((128, 128), f8)
        o[:64, :64] = w
        o[64:, 64:] = w
        return o

    def dup(b):  # [64] -> [128]
        return np.concatenate([b, b])

    ws = {}
    wn1 = np.zeros((6, 128), f8)
    wn1[0:3, 0:64] = g["node_w1"]
    wn1[3:6, 64:128] = g["node_w1"]
    ws["w_node1"] = wn1
    ws["w_node2"] = bd(g["node_w2"])
    e1 = g["edge_w1"]
    wein = np.zeros((8, 128), f8)
    wein[0, 0:64] = e1[0]     # dr0 walker a
    wein[1, 0:64] = e1[1]     # dr1 walker a
    wein[2, 64:128] = e1[0]   # dr0 walker b
    wein[3, 64:128] = e1[1]   # dr1 walker b
    wein[4, 0:64] = e1[3]     # r2 walker a
    wein[5, 64:128] = e1[3]   # r2 walker b
    wein[6, 0:64] = e1[2]     # rr walker a
    wein[7, 64:128] = e1[2]   # rr walker b
    ws["w_ein"] = wein
    ws["w_c"] = bd(w_c)
    ws["w_a0"] = bd(w_a0)
    ws["w_b1"] = bd(w_b1)
    ws["w_a1"] = bd(w_a1)
    ws["w_g0"] = bd(w_g0)
    ws["w_g1"] = bd(w_g1)
    ws["w_top0"] = bd(wtop0)
    ws["w_top1"] = bd(wtop1)
    # hu1 = wt1.T@hv1 = wt1.T@hv0 + wtg.T@accd0 (+folded bias): lets stage3
    # start from accd0 directly instead of the hv1 -> hu1 serial chain
    ws["w_tg"] = bd(w_g0 @ wtop1)
    ws["w_h1"] = bd(g["head_w1"])
    wh2 = np.zeros((128, 4), f8)
    wh2[0:64, 0:2] = w_h2
    wh2[64:128, 2:4] = w_h2
    ws["w_h2"] = wh2
    ws["eye64"] = np.eye(64, dtype=f8)

    bias = np.zeros((128, 12), f8)
    bias[:, 0] = dup(g["node_b1"])
    bias[:, 1] = dup(g["node_b2"])
    bias[:, 2] = dup(g["edge_b1"])
    bias[:, 3] = dup(b_p2)
    bias[:, 4] = dup(b_p3)
    bias[:, 5] = dup(b_p4)
    bias[:, 6] = dup(b_p5)
    bias[:, 7] = dup(g["e2v_b2"][0])
    bias[:, 8] = dup(g["e2v_b2"][1])
    bias[:, 9] = dup(g["head_b1"])
    bias[0:4, 10] = [b_h2[0], b_h2[1], b_h2[0], b_h2[1]]
    ws["biases"] = bias
    return {k: np.ascontiguousarray(v, dtype=np.float32) for k, v in ws.items()}


def _pack_consts(ws, xt, st_):
    """Assemble the bf16 [128, 2244] and fp32 [128, 236] const arrays
    (see _build column maps)."""
    import ml_dtypes

    wb = np.zeros((128, 2244), np.float32)
    wf = np.zeros((128, 236), np.float32)

    def put(dst, col, arr):
        a = np.asarray(arr, np.float32)
        dst[: a.shape[0], col : col + a.shape[1]] = a

    put(wb, 0, ws["w_node1"])
    put(wb, 128, ws["w_node2"])
    put(wb, 256, ws["w_ein"])
    put(wb, 384, ws["w_c"])
    put(wb, 512, ws["w_a0"])
    put(wb, 640, ws["w_b1"])
    put(wb, 768, ws["w_a1"])
    put(wb, 896, ws["w_g0"])
    put(wb, 1024, ws["w_g1"])
    put(wb, 1152, ws["w_top0"])
    put(wb, 1280, ws["w_top1"])
    put(wb, 1408, ws["w_h1"])
    put(wb, 1536, ws["w_h2"])
    put(wb, 1540, ws["eye64"])
    BCl, Nl = xt.shape[0], xt.shape[2]
    nin = np.concatenate([xt, st_], axis=1).reshape(PAIRS, 6, Nl)
    put(wb, 1604, nin.transpose(1, 0, 2).reshape(6, 8 * Nl))
    put(wb, 2116, ws["w_tg"])

    put(wf, 0, ws["biases"])
    put(wf, 12, xt.reshape(BCl, 2, 8, 8).transpose(0, 2, 1, 3).reshape(128, 16))
    put(wf, 28, np.repeat(xt.reshape(BCl, 1, 2 * Nl), 8, axis=1).reshape(128, 2 * Nl))
    put(wf, 156, xt.reshape(128, 16))
    put(wf, 172, np.repeat(xt.reshape(BCl * 2, 1, Nl), 4, axis=1).reshape(128, Nl))
    return wb.astype(ml_dtypes.bfloat16), wf


def kernel(**inputs) -> np.ndarray:
    x = np.asarray(inputs["x"], dtype=np.float32)       # [B, N, D]
    spin = np.asarray(inputs["spin"], dtype=np.float32) # [B, N, 1]
    ws = _prep_weights(inputs)

    if "nc" not in _BUILT:
        _BUILT["nc"] = _build()
    nc = _BUILT["nc"]

    in_maps = []
    for c in range(NCORES):
        xc = x[c * BC : (c + 1) * BC]                     # [16, N, 2]
        sc = spin[c * BC : (c + 1) * BC]                  # [16, N, 1]
        xt = np.ascontiguousarray(xc.transpose(0, 2, 1))  # [16, 2, N]
        st = np.ascontiguousarray(sc.transpose(0, 2, 1))  # [16, 1, N]
        wb, wf = _pack_consts(ws, xt, st)
        in_maps.append({"wpb": wb, "wpf": wf})

    res = run_bass_kernel_spmd(
        nc,
        in_maps,
        core_ids=list(range(NCORES)),
        trace=os.environ.get("BACKFLOW_TRACE", "0") == "1",
    )
    kernel.last_results = res
    out = np.concatenate([r["out_dx"] for r in res.results], axis=0)
    return out.astype(np.float32)



# revision 78
# speedup vs baseline: 1.1025x; 1.1025x over previous
"""BackflowNet GNN message-passing kernel for 8x Trainium2 NeuronCores.

Data-parallel over the walker axis B=128 -> 16 walkers per core, processed as
8 "pairs" (2 walkers block-diag-packed into the 128-partition dim).

Math restructuring (exact, host-side weight folding only):
  he0 = gelu(ein @ ew1 + eb1) @ ew2 + eb2           -> keep g_e = gelu(...)
  layer l: z = hv[:,i]@Wtop + he@Wbot + b1          (he = prev g @ w2 + b2 folded)
           g = gelu(z); he' = g @ w2 + b2
           m = gelu(he' @ e2v_w1 + e2v_b1)          (per-edge)
           hv += (sum_{i!=j} m_i) @ e2v_w2/(N-1) + e2v_b2   (sum moved before w2)
  head: dx = tanh(hv@hw1+hb1) @ (hw2*sp) + hb2*sp;  out = dx - mean_j dx
  hu1 shortcut: hu1 = wt1.T@hv1 = wt1.T@hv0 (early) + (wg0@wt1).T@accd0, with
  hv1's bias folded into the layer-1 gelu bias - stage3 starts straight off
  the aggregation instead of the serial hv1 -> hu1 chain.
Per-edge tensors are feature-major [feat(part), e] with e = j*64 + i
(i = source = innermost so the aggregation is an innermost DVE reduce).

Everything the PE touches is bf16 (weights, activation streams): 1 col/cycle
like f32r but with FWL fast weight loads, half the SBUF, and standalone
Ldweights+Matmult lowering whose redundant weight reloads _dedup_ldw elides
at the BIR level (walrus --enable-ldw-opt rejects standalone InstLdweights).
PSUM accumulates fp32; biases/x-packs stay fp32. The i<-j "hu" broadcast-add
is folded into the PE as an identity-broadcast accumulation (eye_bc) rather
than a DVE add into PSUM (which runs 1x and sat on the critical path).

Scheduling: 5 pipeline stages (DMA prefetch / ein+wc / agg0 / wb1 / agg1+head)
emitted as block-granular thunks, round-robined RRN=2 at a time across the
4-5 pairs in flight so each engine's queue interleaves independent work; the
last pair's stage3/4 are fused block-wise to shorten the drain tail.
"""

import math
import os

import numpy as np

import concourse.bass as bass
import concourse.mybir as mybir
import concourse.tile as tile
import concourse.bass_utils as _bu
from concourse.bass_utils import run_bass_kernel_spmd

if os.environ.get("BACKFLOW_LDW_OPT", "0") == "1" and not getattr(_bu, "_ldw_patched", False):
    _bu._ldw_patched = True
    _orig_run = _bu.run_command

    def _run(cmd, cwd=None):
        if cmd and "walrus_driver" in cmd[0]:
            cmd = [c if c != "--enable-ldw-opt=false" else "--enable-ldw-opt=true"
                   for c in cmd]
        return _orig_run(cmd, cwd=cwd)

    _bu.run_command = _run

NCORES = 8
B, N, D = 128, 64, 2
H = 64
M = 64
BC = B // NCORES          # walkers per core
PAIRS = BC // 2           # walker pairs per core
E = N * N                 # edges (incl. diagonal) per walker
SUB = int(os.environ.get("BACKFLOW_SUB", "512"))  # matmul moving free dim
                          # (bf16 moving operands allow up to 1024)
# gelu/psum blocks: uneven (3 ACT ops per pass instead of 4 cuts the
# per-op (N+352)/1.2 ns ACT overhead)
_B = int(os.environ.get("BACKFLOW_BLK", "1024"))
BLOCKS = ([(0, 1536), (1536, 1536), (3072, 1024)] if _B == 1536
          else [(0, 1024), (1024, 1024), (2048, 1024), (3072, 1024)])
PBLK = _B                 # psum tile width
# emission groups: pairs of blocks share one weight-load train per src
# (measured worse: holding two PSUM tiles per group starves the pipeline)
BPAIR = os.environ.get("BACKFLOW_BPAIR", "0") == "1"
BGROUPS = ([(0, 1), (2, 3)] if (len(BLOCKS) == 4 and BPAIR)
           else [(b,) for b in range(len(BLOCKS))])
F32 = mybir.dt.float32
F32R = mybir.dt.float32r
BF16 = mybir.dt.bfloat16
AF = mybir.ActivationFunctionType
AX = mybir.AxisListType

HU_DVE = os.environ.get("BACKFLOW_HU_DVE", "0") == "1"
# agg-pass gelu outputs + row-sum accumulators in bf16: DVE tensor_reduce
# runs in 2x mode (16-bit, step 1) halving the aggregation reduce cost
BF16_AGG = os.environ.get("BACKFLOW_BF16_AGG", "1") == "1"
GB_DT = BF16 if BF16_AGG else F32R

_BUILT = {}


def _legalize_sync(bir_bytes):
    """Walrus on this toolchain encodes at most one semaphore wait per
    engine instruction (none on DMA queue entries). Tile attaches as many
    waits as deps require, so spill the surplus into standalone
    EventSemaphore instructions on the same engine, placed just before."""
    import json as _json

    d = _json.loads(bir_bytes)
    n = [0]
    # salt the generated names so compile-flag-only changes (which do not
    # otherwise alter the BIR/HLO bytes) miss the remote executable cache
    salt = "L" + os.environ.get("BACKFLOW_LDW_OPT", "0") + os.environ.get(
        "BACKFLOW_SALT", ""
    )

    def fix_block(bb):
        insts = bb.get("instructions")
        if not insts:
            return
        out = []
        for ins in insts:
            si = ins.get("sync_info")
            waits = (si or {}).get("on_wait") or []
            opc = ins.get("opcode", "")
            if opc == "EventSemaphore":
                allowed = 1
            elif opc.startswith("DMA") or ins.get("queue"):
                allowed = 0
            else:
                allowed = 1
            if len(waits) > allowed:
                keep, spill = waits[:allowed], waits[allowed:]
                for w in spill:
                    n[0] += 1
                    out.append({
                        "debug": ins.get("debug", 0),
                        "engine": ins["engine"],
                        "ins": [],
                        "outs": [],
                        "name": f"evw{salt}-{n[0]}",
                        "opcode": "EventSemaphore",
                        "sync_info": {"on_update": [], "on_wait": [w]},
                    })
                si["on_wait"] = keep
            out.append(ins)
        bb["instructions"] = out

    def walk(obj):
        if isinstance(obj, dict):
            if "instructions" in obj:
                fix_block(obj)
            else:
                for v in obj.values():
                    walk(v)
        elif isinstance(obj, list):
            for v in obj:
                walk(v)

    walk(d)
    return _json.dumps(d).encode()


def _dedup_ldw(bir_bytes):
    """bf16 matmuls lower to standalone Ldweights+Matmult pairs with one
    weight load per matmul. The PE array retains stationary weights across
    matmuls, so consecutive PE loads of the identical weight AP are
    redundant — drop them (walrus's --enable-ldw-opt rejects standalone
    InstLdweights, so do it here). Dropped loads' semaphore waits are
    spilled as standalone PE EventSemaphores to preserve ordering."""
    import json as _json

    d = _json.loads(bir_bytes)
    n = [0]

    def fix_block(bb):
        insts = bb.get("instructions")
        if not insts:
            return
        out = []
        last_sig = None
        pending = []
        for ins in insts:
            if ins.get("engine") == "PE":
                opc = ins.get("opcode", "")
                if opc == "Ldweights":
                    sig = _json.dumps(
                        [
                            ins.get("ins"),
                            ins.get("tile_position"),
                            ins.get("tile_size"),
                            ins.get("perf_mode"),
                            ins.get("is_transpose"),
                        ],
                        sort_keys=True,
                    )
                    si = ins.get("sync_info") or {}
                    if sig == last_sig and not si.get("on_update"):
                        pending.extend(si.get("on_wait") or [])
                        continue
                    last_sig = sig
                elif opc not in ("Matmult", "EventSemaphore"):
                    last_sig = None
                if pending:
                    for w in pending:
                        n[0] += 1
                        out.append({
                            "debug": ins.get("debug", 0),
                            "engine": "PE",
                            "ins": [],
                            "outs": [],
                            "name": f"ldww-{n[0]}",
                            "opcode": "EventSemaphore",
                            "sync_info": {"on_update": [], "on_wait": [w]},
                        })
                    pending = []
            out.append(ins)
        bb["instructions"] = out

    def walk(obj):
        if isinstance(obj, dict):
            if "instructions" in obj:
                fix_block(obj)
            else:
                for v in obj.values():
                    walk(v)
        elif isinstance(obj, list):
            for v in obj:
                walk(v)

    walk(d)
    return _json.dumps(d).encode()


def _build():
    nc = bass.Bass(
        "TRN2", target_bir_lowering=False, debug=False, enable_asserts=False
    )

    # Constants in two packed dram tensors: bf16 weights/streams (FWL-fast
    # weight loads, half the SBUF/DMA), fp32 biases + x packs (ACT bias APs
    # and fp32 phase-0 DVE math).
    # wpb (bf16) column map:
    #  0:128    wn1 [6,128]       128:256  wn2        256:384  wein [8,128]
    #  384:512  wc                512:640  wa0        640:768  wb1
    #  768:896  wa1               896:1024 wg0        1024:1152 wg1
    #  1152:1280 wt0              1280:1408 wt1       1408:1536 wh1
    #  1536:1540 wh2 [128,4]      1540:1604 eye [64,64]
    #  1604:2116 nin [6,512]      2116:2244 wtg
    # wpf (fp32) column map:
    #  0:12 biases  12:28 xp_lhs  28:156 xp_rhs  156:172 xq_lhs  172:236 xq_rhs
    #  236:238 head bias [2(q), 2(d)]
    CB, CF = 2244, 238
    wpack_bf = nc.dram_tensor("wpb", [128, CB], BF16, kind="ExternalInput").ap()
    wpack_f32 = nc.dram_tensor("wpf", [128, CF], F32, kind="ExternalInput").ap()
    out_dx = nc.dram_tensor("out_dx", [BC, N, D], F32, kind="ExternalOutput").ap()

    with tile.TileContext(nc) as tc:
        with (
            tc.tile_pool(name="consts", bufs=1) as consts,
            tc.tile_pool(name="feat", bufs=1) as feat,
            tc.tile_pool(name="big", bufs=12) as big,
            tc.tile_pool(name="gblk", bufs=10) as gblk,
            tc.tile_pool(name="small", bufs=24) as small,
            tc.tile_pool(name="swide", bufs=4) as swide,
            tc.tile_pool(name="pbig", bufs=(2 if _B == 1536 else 3),
                         space="PSUM") as pbig,
            tc.tile_pool(name="psmall", bufs=2, space="PSUM") as psmall,
        ):
            wpb = consts.tile([128, CB], BF16, name="wpb", tag="wpb")
            wpf = consts.tile([128, CF], F32, name="wpf", tag="wpf")
            # phase-0 inputs + node-MLP columns land first on their own
            # queue so compute starts before the big weight block finishes
            nc.gpsimd.dma_start(out=wpf, in_=wpack_f32)
            nc.gpsimd.dma_start(out=wpb[:, 1604:CB], in_=wpack_bf[:, 1604:CB])
            nc.sync.dma_start(out=wpb[:, 0:1604], in_=wpack_bf[:, 0:1604])
            wn1 = wpb[0:6, 0:128]
            wn2 = wpb[:, 128:256]
            wein = wpb[0:8, 256:384]
            wc = wpb[:, 384:512]
            wa0 = wpb[:, 512:640]
            wb1 = wpb[:, 640:768]
            wa1 = wpb[:, 768:896]
            wg0 = wpb[:, 896:1024]
            wg1 = wpb[:, 1024:1152]
            wt0 = wpb[:, 1152:1280]
            wt1 = wpb[:, 1280:1408]
            wh1 = wpb[:, 1408:1536]
            wh2 = wpb[:, 1536:1540]
            eye_t = wpb[0:64, 1540:1604]
            nin = wpb[0:6, 1604:2116]
            wtg = wpb[:, 2116:2244]
            bia = wpf[:, 0:12]
            xpl = wpf[:, 12:28]
            xpr = wpf[:, 28:156]
            xql = wpf[:, 156:172]
            xqr = wpf[:, 172:236]
            bh2 = wpf[0:2, 236:238]
            eps_t = consts.tile([128, 1], F32, name="eps_t", tag="eps_t")
            nc.vector.memset(eps_t, 1e-12)

            # identity-broadcast rhs: rhs[k,(j,i)] = eye[k,i], j in 8-block
            eye_bc = eye_t.unsqueeze(1).broadcast_to([64, SUB // N, 64])


            import contextlib
            rep = int(os.environ.get("BACKFLOW_REPEAT", "1"))
            loop_cm = tc.For_i(0, rep, 1) if rep > 1 else contextlib.nullcontext()
            with loop_cm:
                # -------- phase 0: dr / r2 / rr in packed layouts ---------------
                # (w,d,jq2)-packed dr for the ein rows (contiguous per pair)
                dre_pk = feat.tile([128, 1024], BF16, name="dre_pk", tag="drepk")
                nc.vector.tensor_sub(
                    out=dre_pk.rearrange("p (j i) -> p j i", i=N),
                    in0=xql.unsqueeze(2).broadcast_to([128, 16, N]),
                    in1=xqr.unsqueeze(1).broadcast_to([128, 16, N]),
                )
                # (w,jq)-packed dr for r2/rr
                dr_pk = feat.tile([128, 1024], F32, name="dr_pk", tag="drpk")
                nc.vector.tensor_sub(
                    out=dr_pk.rearrange("p (d j i) -> p d j i", d=2, i=N),
                    in0=xpl.rearrange("p (d j) -> p d j", d=2)
                        .unsqueeze(3).broadcast_to([128, 2, 8, N]),
                    in1=xpr.rearrange("p (d i) -> p d i", d=2)
                        .unsqueeze(2).broadcast_to([128, 2, 8, N]),
                )
                sq_pk = feat.tile([128, 1024], F32, name="sq_pk", tag="sqpk")
                nc.vector.tensor_mul(out=sq_pk, in0=dr_pk, in1=dr_pk)
                sqv = sq_pk.rearrange("p (d f) -> p d f", d=2)
                r2_pk = feat.tile([128, 512], BF16, name="r2_pk", tag="r2pk")
                nc.vector.tensor_add(out=r2_pk, in0=sqv[:, 0, :], in1=sqv[:, 1, :])
                rr_pk = feat.tile([128, 512], BF16, name="rr_pk", tag="rrpk")
                sqrt_inst = nc.scalar.activation(
                    out=rr_pk, in_=r2_pk, func=AF.Sqrt, bias=eps_t, scale=1.0
                )

                # -------- batched node MLP (all 8 pairs) ------------------------
                zn = psmall.tile([128, 8 * N], F32, name="zn", tag="ps_s")
                zn_inst = nc.tensor.matmul(zn, wn1, nin)
                from concourse.tile_rust import add_dep_helper
                add_dep_helper(zn_inst.ins, sqrt_inst.ins,
                               reason="keep sqrt first in ACT stream (table set)")
                gn = swide.tile([128, 8 * N], BF16, name="gn", tag="sw")
                nc.scalar.activation(
                    out=gn, in_=zn, func=AF.Gelu, bias=bia[:, 0:1], scale=1.0
                )
                hv0p = psmall.tile([128, 8 * N], F32, name="hv0p", tag="ps_s")
                nc.tensor.matmul(hv0p, wn2, gn)
                hv0a = swide.tile([128, 8 * N], BF16, name="hv0a", tag="sw")
                nc.vector.tensor_scalar_add(out=hv0a, in0=hv0p, scalar1=bia[:, 1:2])

                # -------- software-pipelined per-pair stages --------------------
                st = [dict() for _ in range(PAIRS)]

                def edge_pass_group(bls, dsts, srcs, gelu_bias, hu_m=None,
                                    post=None):
                    """Emit a group of blocks sharing one weight-load train
                    per src (srcs outer, blocks+chunks inner: consecutive
                    same-weight matmuls keep the PE array's stationary
                    weights, and the bf16 standalone-LDW dedup drops the
                    reloads). `post(bi, bl)` runs per block after its gelu
                    (reduce/sub for agg passes)."""
                    pss = []
                    for _ in bls:
                        pss.append(
                            pbig.tile([128, PBLK], F32, name="ps", tag="ps_b")
                        )
                    for k, (lhsT, rhs_fn) in enumerate(srcs):
                        for bi, bl in enumerate(bls):
                            lo0, blen = BLOCKS[bl]
                            for s in range(blen // SUB):
                                lo = lo0 + s * SUB
                                out_sl = pss[bi][:, s * SUB : (s + 1) * SUB]
                                nc.tensor.matmul(
                                    out_sl,
                                    lhsT,
                                    rhs_fn(lo),
                                    start=(k == 0),
                                    stop=(k == len(srcs) - 1),
                                )
                    for bi, bl in enumerate(bls):
                        lo0, blen = BLOCKS[bl]
                        ps = pss[bi]
                        if hu_m is not None:
                            psv = ps[:, 0:blen].rearrange("p (j i) -> p j i", i=N)
                            nc.vector.tensor_add(
                                out=psv, in0=psv,
                                in1=hu_m.unsqueeze(1).broadcast_to(
                                    [128, blen // N, N]
                                ),
                            )
                        nc.scalar.activation(
                            out=dsts[bi][:, 0:blen], in_=ps[:, 0:blen],
                            func=AF.Gelu, bias=gelu_bias, scale=1.0,
                        )
                        if post is not None:
                            post(bi, bl)

                def edge_pass_blk(bl, dst, srcs, gelu_bias, hu_m=None):
                    edge_pass_group([bl], [dst], srcs, gelu_bias, hu_m=hu_m)

                def agg_group(bls, src_g, wagg, bias_col, acc, accd):
                    gbs = [
                        gblk.tile([128, PBLK], GB_DT, name="gb", tag="gb")
                        for _ in bls
                    ]

                    def post(bi, bl):
                        lo0, blen = BLOCKS[bl]
                        gb = gbs[bi]
                        jlo = lo0 // N
                        jb = blen // N
                        with nc.allow_low_precision(
                            reason="bf16 row-sum: 2e-2 rel-err budget"
                        ):
                            nc.vector.reduce_sum(
                                out=acc[:, jlo : jlo + jb],
                                in_=gb[:, 0:blen].rearrange(
                                    "p (j i) -> p j i", i=N
                                ),
                                axis=AX.X,
                            )
                        diag = gb[:, jlo : jlo + (jb - 1) * (N + 1) + 1 : N + 1]
                        nc.vector.tensor_sub(
                            out=accd[:, jlo : jlo + jb],
                            in0=acc[:, jlo : jlo + jb],
                            in1=diag,
                        )

                    edge_pass_group(
                        bls, gbs,
                        [(wagg, lambda lo: src_g[:, lo : lo + SUB])],
                        bias_col, post=post,
                    )

                # Each stage returns a list of block-granular thunks; the
                # scheduler round-robins the lists so every engine's queue
                # interleaves independent work from 4-5 pairs at block
                # granularity (one stage's stall no longer head-of-line
                # blocks the whole step).
                def stage0(p):
                    # prefetch: per-pair edge-input DMAs one pipeline step
                    # early so stage1's matmuls never head-of-line block PE
                    def t_dma():
                        s = st[p]
                        ein = big.tile([8, E], BF16, name="ein", tag="stream")
                        s["ein"] = ein
                        nc.gpsimd.dma_start(
                            out=ein[0:4, :], in_=dre_pk[16 * p : 16 * p + 16, :]
                        )
                        nc.gpsimd.dma_start(
                            out=ein[4:6, :], in_=r2_pk[16 * p : 16 * p + 16, :]
                        )
                        nc.gpsimd.dma_start(
                            out=ein[6:8, :], in_=rr_pk[16 * p : 16 * p + 16, :]
                        )
                    return [t_dma]

                def stage1(p):
                    s = st[p]

                    def t_setup():
                        hv0 = hv0a[:, p * N : (p + 1) * N]
                        s["hv0"] = hv0
                        if HU_DVE:
                            hu0p = psmall.tile([128, 64], F32, name="hu0p", tag="ps_s")
                            nc.tensor.matmul(hu0p, wt0, hv0)
                            hu0 = small.tile([128, 64], F32, name="hu0", tag="sm")
                            nc.vector.tensor_copy(out=hu0, in_=hu0p)
                            # early half of hu1: wt1.T @ hv0 (the accd0-
                            # dependent half lands in stage2 via wtg)
                            hu1ap = psmall.tile([128, 64], F32, name="hu1ap", tag="ps_s")
                            nc.tensor.matmul(hu1ap, wt1, hv0)
                            hu1a = small.tile([128, 64], F32, name="hu1a", tag="sm")
                            nc.vector.tensor_copy(out=hu1a, in_=hu1ap)
                        else:
                            hu0p = psmall.tile([64, 128], F32, name="hu0p", tag="ps_s")
                            nc.tensor.matmul(hu0p, hv0, wt0)
                            hu0 = small.tile([64, 128], BF16, name="hu0", tag="smh")
                            nc.vector.tensor_copy(out=hu0, in_=hu0p)
                            hu1ap = psmall.tile([64, 128], F32, name="hu1ap", tag="ps_s")
                            nc.tensor.matmul(hu1ap, hv0, wt1)
                            hu1a = small.tile([64, 128], F32, name="hu1a", tag="sm")
                            nc.vector.tensor_copy(out=hu1a, in_=hu1ap)
                        s["hu0"] = hu0
                        s["hu1a"] = hu1a
                        s["g1"] = big.tile([128, E], BF16, name="g1", tag="stream")
                        s["gebs"] = {}

                    def t_ein(gi):
                        bls = BGROUPS[gi]
                        gebs = []
                        for bl in bls:
                            # own tag: under RR interleave, 'gb'-tag tiles
                            # from other stages would recycle geb's buffer
                            # before the wc pass (a later PE instruction)
                            # reads it — same-queue wait inversion = deadlock
                            geb = gblk.tile([128, PBLK], BF16, name="geb",
                                            tag="geb", bufs=5)
                            s["gebs"][bl] = geb
                            gebs.append(geb)
                        ein = s["ein"]
                        edge_pass_group(
                            list(bls), gebs,
                            [(wein, lambda lo: ein[:, lo : lo + SUB])],
                            bia[:, 2:3],
                        )

                    def t_wc(gi):
                        bls = BGROUPS[gi]
                        g1 = s["g1"]
                        dsts = []
                        for bl in bls:
                            lo0, blen = BLOCKS[bl]
                            dsts.append(g1[:, lo0 : lo0 + blen])

                        def geb_rhs(lo):
                            bl = next(
                                b for b in bls
                                if BLOCKS[b][0] <= lo < BLOCKS[b][0] + BLOCKS[b][1]
                            )
                            lo0 = BLOCKS[bl][0]
                            return s["gebs"][bl][:, lo - lo0 : lo - lo0 + SUB]

                        if HU_DVE:
                            edge_pass_group(
                                list(bls), dsts,
                                [(wc, geb_rhs)],
                                bia[:, 3:4], hu_m=s["hu0"],
                            )
                        else:
                            edge_pass_group(
                                list(bls), dsts,
                                [
                                    (wc, geb_rhs),
                                    (s["hu0"], lambda lo: eye_bc),
                                ],
                                bia[:, 3:4],
                            )

                    import functools
                    return (
                        [t_setup]
                        + [functools.partial(t_ein, g) for g in range(len(BGROUPS))]
                        + [functools.partial(t_wc, g) for g in range(len(BGROUPS))]
                    )

                def stage2(p):
                    s = st[p]

                    def t_setup():
                        s["acc0"] = small.tile([128, N], GB_DT, name="acc0", tag="smh")
                        s["accd0"] = small.tile([128, N], BF16, name="accd0", tag="smh")

                    def t_blk(gi):
                        agg_group(list(BGROUPS[gi]), s["g1"], wa0, bia[:, 4:5],
                                  s["acc0"], s["accd0"])

                    def t_tail():
                        accd0 = s["accd0"]
                        # critical: hu1 = hu1a + wtg.T@accd0 feeds stage3
                        if HU_DVE:
                            hu1cp = psmall.tile([128, 64], F32, name="hu1cp", tag="ps_s")
                            nc.tensor.matmul(hu1cp, wtg, accd0)
                            hu1 = small.tile([128, 64], F32, name="hu1", tag="sm")
                        else:
                            hu1cp = psmall.tile([64, 128], F32, name="hu1cp", tag="ps_s")
                            nc.tensor.matmul(hu1cp, accd0, wtg)
                            hu1 = small.tile([64, 128], BF16, name="hu1", tag="smh")
                        nc.vector.tensor_add(out=hu1, in0=hu1cp, in1=s["hu1a"])
                        s["hu1"] = hu1
                        # off the stage3 critical path: hv1 (used by stage4)
                        u0p = psmall.tile([128, N], F32, name="u0p", tag="ps_s")
                        nc.tensor.matmul(u0p, wg0, accd0)
                        hv1 = small.tile([128, N], F32R, name="hv1", tag="sm")
                        nc.vector.scalar_tensor_tensor(
                            out=hv1, in0=u0p, scalar=bia[:, 7:8], in1=s["hv0"],
                            op0=mybir.AluOpType.add, op1=mybir.AluOpType.add,
                        )
                        s["hv1"] = hv1

                    import functools
                    return (
                        [t_setup]
                        + [functools.partial(t_blk, g) for g in range(len(BGROUPS))]
                        + [t_tail]
                    )

                def stage3(p):
                    s = st[p]

                    def t_setup():
                        s["g3"] = big.tile([128, E], BF16, name="g3", tag="stream")

                    def t_blk(gi):
                        bls = BGROUPS[gi]
                        g1, g3 = s["g1"], s["g3"]
                        dsts = []
                        for bl in bls:
                            lo0, blen = BLOCKS[bl]
                            dsts.append(g3[:, lo0 : lo0 + blen])
                        if HU_DVE:
                            edge_pass_group(
                                list(bls), dsts,
                                [(wb1, lambda lo: g1[:, lo : lo + SUB])],
                                bia[:, 5:6], hu_m=s["hu1"],
                            )
                        else:
                            edge_pass_group(
                                list(bls), dsts,
                                [
                                    (wb1, lambda lo: g1[:, lo : lo + SUB]),
                                    (s["hu1"], lambda lo: eye_bc),
                                ],
                                bia[:, 5:6],
                            )

                    import functools
                    return [t_setup] + [
                        functools.partial(t_blk, g) for g in range(len(BGROUPS))
                    ]

                def stage4(p):
                    s = st[p]

                    def t_setup():
                        s["acc1"] = small.tile([128, N], GB_DT, name="acc1", tag="smh")
                        s["accd1"] = small.tile([128, N], BF16, name="accd1", tag="smh")

                    def t_blk(gi):
                        agg_group(list(BGROUPS[gi]), s["g3"], wa1, bia[:, 6:7],
                                  s["acc1"], s["accd1"])

                    def t_tail():
                        accd1 = s["accd1"]
                        u1p = psmall.tile([128, N], F32, name="u1p", tag="ps_s")
                        nc.tensor.matmul(u1p, wg1, accd1)
                        hv2 = small.tile([128, N], BF16, name="hv2", tag="smh")
                        nc.vector.scalar_tensor_tensor(
                            out=hv2, in0=u1p, scalar=bia[:, 8:9], in1=s["hv1"],
                            op0=mybir.AluOpType.add, op1=mybir.AluOpType.add,
                        )
                        thp = psmall.tile([128, N], F32, name="thp", tag="ps_s")
                        nc.tensor.matmul(thp, wh1, hv2)
                        th = small.tile([128, N], BF16, name="th", tag="smh")
                        nc.scalar.activation(
                            out=th, in_=thp, func=AF.Tanh, bias=bia[:, 9:10], scale=1.0
                        )
                        # head output directly in per-walker (n,d)-interleaved
                        # rows: two matmuls write even/odd psum columns, so
                        # the output DMA is two contiguous 512B rows instead
                        # of scattered 4-byte writes (whose completion drain
                        # used to cost >10us at the kernel tail)
                        dxp = psmall.tile([2, N * D], F32, name="dxp", tag="ps_s")
                        dxpv = dxp.rearrange("p (n d) -> p d n", d=2)
                        nc.tensor.matmul(dxpv[:, 0, :], wh2[:, 0:2], th)
                        nc.tensor.matmul(dxpv[:, 1, :], wh2[:, 2:4], th)
                        dx = small.tile([2, N * D], F32, name="dx", tag="sm")
                        dxv = dx.rearrange("p (n d) -> p d n", d=2)
                        nc.vector.tensor_add(
                            out=dxv, in0=dxpv,
                            in1=bh2.unsqueeze(2).broadcast_to([2, 2, N]),
                        )
                        msum = small.tile([2, 2], F32, name="msum", tag="sm1")
                        nc.vector.reduce_sum(out=msum, in_=dxv, axis=AX.X)
                        negm = small.tile([2, 2], F32, name="negm", tag="sm1")
                        nc.vector.tensor_scalar_mul(
                            out=negm, in0=msum, scalar1=-1.0 / N
                        )
                        dxf = small.tile([2, N * D], F32, name="dxf", tag="sm")
                        nc.vector.tensor_add(
                            out=dxf.rearrange("p (n d) -> p d n", d=2), in0=dxv,
                            in1=negm.unsqueeze(2).broadcast_to([2, 2, N]),
                        )
                        nc.sync.dma_start(
                            out=out_dx[2 * p : 2 * p + 2].rearrange(
                                "b n d -> b (n d)"
                            ),
                            in_=dxf,
                        )

                    import functools
                    return (
                        [t_setup]
                        + [functools.partial(t_blk, g) for g in range(len(BGROUPS))]
                        + [t_tail]
                    )

                stages = [stage0, stage1, stage2, stage3, stage4]
                order = [int(c) for c in os.environ.get("BACKFLOW_ORDER", "01243")]
                # round-robin grain: thunks popped per stage per cycle.
                # 0 = no interleave (whole stage at once)
                RRN = int(os.environ.get("BACKFLOW_RRN", "2"))
                for t in range(PAIRS + len(stages) - 1):
                    lists = []
                    for si in order:
                        p = t - si
                        if 0 <= p < PAIRS:
                            if si == 3 and p == PAIRS - 1:
                                # drain tail: fuse the last pair's stage3/4
                                # block-wise so they pipeline instead of
                                # running back-to-back alone
                                l3, l4 = stage3(p), stage4(p)
                                fused = [l3.pop(0), l3.pop(0), l4.pop(0)]
                                while l3 or l4:
                                    if l3:
                                        fused.append(l3.pop(0))
                                    if l4:
                                        fused.append(l4.pop(0))
                                lists.append(fused)
                            elif si == 4 and p == PAIRS - 1:
                                pass
                            else:
                                lists.append(stages[si](p))
                    if RRN > 0:
                        while any(lists):
                            for lst in lists:
                                for _ in range(min(RRN, len(lst))):
                                    lst.pop(0)()
                    else:
                        for lst in lists:
                            for th in lst:
                                th()

    patched = _legalize_sync(_dedup_ldw(nc.to_json_bytes()))
    nc.to_json_bytes = lambda: patched
    return nc


def _prep_weights(inputs):
    f8 = np.float64
    g = {k: np.asarray(v, dtype=f8) for k, v in inputs.items()}
    inv = 1.0 / (N - 1)

    wtop0 = g["v2e_w1"][0][:H]
    wbot0 = g["v2e_w1"][0][H:]
    wtop1 = g["v2e_w1"][1][:H]
    wbot1 = g["v2e_w1"][1][H:]

    w_c = g["edge_w2"] @ wbot0
    b_p2 = g["edge_b2"] @ wbot0 + g["v2e_b1"][0]
    w_a0 = g["v2e_w2"][0] @ g["e2v_w1"][0]
    b_p3 = g["v2e_b2"][0] @ g["e2v_w1"][0] + g["e2v_b1"][0]
    w_b1 = g["v2e_w2"][0] @ wbot1
    # hv1's bias (e2v_b2[0]) contribution to hu1 = hv1.T @ wtop1 is constant
    # over i, so it folds into the layer-1 gelu bias column
    b_p4 = g["v2e_b2"][0] @ wbot1 + g["v2e_b1"][1] + g["e2v_b2"][0] @ wtop1
    w_a1 = g["v2e_w2"][1] @ g["e2v_w1"][1]
    b_p5 = g["v2e_b2"][1] @ g["e2v_w1"][1] + g["e2v_b1"][1]
    w_g0 = g["e2v_w2"][0] * inv
    w_g1 = g["e2v_w2"][1] * inv
    sp = float(np.log1p(np.exp(g["scale"][0])))
    w_h2 = g["head_w2"] * sp
    b_h2 = g["head_b2"] * sp

    def bd(w):  # [64,64] -> [128,128] block-diag
        o = np.zeros((128, 128), f8)
        o[:64, :64] = w
        o[64:, 64:] = w
        return o

    def dup(b):  # [64] -> [128]
        return np.concatenate([b, b])

    ws = {}
    wn1 = np.zeros((6, 128), f8)
    wn1[0:3, 0:64] = g["node_w1"]
    wn1[3:6, 64:128] = g["node_w1"]
    ws["w_node1"] = wn1
    ws["w_node2"] = bd(g["node_w2"])
    e1 = g["edge_w1"]
    wein = np.zeros((8, 128), f8)
    wein[0, 0:64] = e1[0]     # dr0 walker a
    wein[1, 0:64] = e1[1]     # dr1 walker a
    wein[2, 64:128] = e1[0]   # dr0 walker b
    wein[3, 64:128] = e1[1]   # dr1 walker b
    wein[4, 0:64] = e1[3]     # r2 walker a
    wein[5, 64:128] = e1[3]   # r2 walker b
    wein[6, 0:64] = e1[2]     # rr walker a
    wein[7, 64:128] = e1[2]   # rr walker b
    ws["w_ein"] = wein
    ws["w_c"] = bd(w_c)
    ws["w_a0"] = bd(w_a0)
    ws["w_b1"] = bd(w_b1)
    ws["w_a1"] = bd(w_a1)
    ws["w_g0"] = bd(w_g0)
    ws["w_g1"] = bd(w_g1)
    ws["w_top0"] = bd(wtop0)
    ws["w_top1"] = bd(wtop1)
    # hu1 = wt1.T@hv1 = wt1.T@hv0 + wtg.T@accd0 (+folded bias): lets stage3
    # start from accd0 directly instead of the hv1 -> hu1 serial chain
    ws["w_tg"] = bd(w_g0 @ wtop1)
    ws["w_h1"] = bd(g["head_w1"])
    # head output columns grouped by d: [a-d0, b-d0 | a-d1, b-d1] so the two
    # per-d matmuls use contiguous stationary slices and write even/odd
    # interleaved psum columns -> contiguous per-walker output rows
    wh2 = np.zeros((128, 4), f8)
    wh2[0:64, 0] = w_h2[:, 0]
    wh2[64:128, 1] = w_h2[:, 0]
    wh2[0:64, 2] = w_h2[:, 1]
    wh2[64:128, 3] = w_h2[:, 1]
    ws["w_h2"] = wh2
    ws["eye64"] = np.eye(64, dtype=f8)

    bias = np.zeros((128, 12), f8)
    bias[:, 0] = dup(g["node_b1"])
    bias[:, 1] = dup(g["node_b2"])
    bias[:, 2] = dup(g["edge_b1"])
    bias[:, 3] = dup(b_p2)
    bias[:, 4] = dup(b_p3)
    bias[:, 5] = dup(b_p4)
    bias[:, 6] = dup(b_p5)
    bias[:, 7] = dup(g["e2v_b2"][0])
    bias[:, 8] = dup(g["e2v_b2"][1])
    bias[:, 9] = dup(g["head_b1"])
    bias[0:4, 10] = [b_h2[0], b_h2[1], b_h2[0], b_h2[1]]
    ws["biases"] = bias
    return {k: np.ascontiguousarray(v, dtype=np.float32) for k, v in ws.items()}


def _pack_consts(ws, xt, st_):
    """Assemble the bf16 [128, 2244] and fp32 [128, 236] const arrays
    (see _build column maps)."""
    import ml_dtypes

    wb = np.zeros((128, 2244), np.float32)
    wf = np.zeros((128, 238), np.float32)

    def put(dst, col, arr):
        a = np.asarray(arr, np.float32)
        dst[: a.shape[0], col : col + a.shape[1]] = a

    put(wb, 0, ws["w_node1"])
    put(wb, 128, ws["w_node2"])
    put(wb, 256, ws["w_ein"])
    put(wb, 384, ws["w_c"])
    put(wb, 512, ws["w_a0"])
    put(wb, 640, ws["w_b1"])
    put(wb, 768, ws["w_a1"])
    put(wb, 896, ws["w_g0"])
    put(wb, 1024, ws["w_g1"])
    put(wb, 1152, ws["w_top0"])
    put(wb, 1280, ws["w_top1"])
    put(wb, 1408, ws["w_h1"])
    put(wb, 1536, ws["w_h2"])
    put(wb, 1540, ws["eye64"])
    BCl, Nl = xt.shape[0], xt.shape[2]
    nin = np.concatenate([xt, st_], axis=1).reshape(PAIRS, 6, Nl)
    put(wb, 1604, nin.transpose(1, 0, 2).reshape(6, 8 * Nl))
    put(wb, 2116, ws["w_tg"])

    put(wf, 0, ws["biases"])
    # head bias as a [2, 2] (q, d) block for the interleaved head layout
    bh2 = ws["biases"][0:2, 10:11]  # rows = [b_h2[0], b_h2[1]]
    put(wf, 236, np.tile(bh2.reshape(1, 2), (2, 1)))
    put(wf, 12, xt.reshape(BCl, 2, 8, 8).transpose(0, 2, 1, 3).reshape(128, 16))
    put(wf, 28, np.repeat(xt.reshape(BCl, 1, 2 * Nl), 8, axis=1).reshape(128, 2 * Nl))
    put(wf, 156, xt.reshape(128, 16))
    put(wf, 172, np.repeat(xt.reshape(BCl * 2, 1, Nl), 4, axis=1).reshape(128, Nl))
    return wb.astype(ml_dtypes.bfloat16), wf


def kernel(**inputs) -> np.ndarray:
    x = np.asarray(inputs["x"], dtype=np.float32)       # [B, N, D]
    spin = np.asarray(inputs["spin"], dtype=np.float32) # [B, N, 1]
    ws = _prep_weights(inputs)

    if "nc" not in _BUILT:
        _BUILT["nc"] = _build()
    nc = _BUILT["nc"]

    in_maps = []
    for c in range(NCORES):
        xc = x[c * BC : (c + 1) * BC]                     # [16, N, 2]
        sc = spin[c * BC : (c + 1) * BC]                  # [16, N, 1]
        xt = np.ascontiguousarray(xc.transpose(0, 2, 1))  # [16, 2, N]
        st = np.ascontiguousarray(sc.transpose(0, 2, 1))  # [16, 1, N]
        wb, wf = _pack_consts(ws, xt, st)
        in_maps.append({"wpb": wb, "wpf": wf})

    res = run_bass_kernel_spmd(
        nc,
        in_maps,
        core_ids=list(range(NCORES)),
        trace=os.environ.get("BACKFLOW_TRACE", "0") == "1",
    )
    kernel.last_results = res
    out = np.concatenate([r["out_dx"] for r in res.results], axis=0)
    return out.astype(np.float32)

